# revision 2
# baseline (speedup 1.0000x reference)
"""Trainium2 Bass kernel for nn_ContextAttention (two-pass label attention).

Reference computation (fp32):
    K = elu(K_w @ H + K_b)        # [B,S,D] keys,  per-position linear over channels
    V = elu(V_w @ H + V_b)        # [B,S,D]
    E1 = Q_w @ K^T * SCALE        # [B,L,S]
    A1 = softmax(E1, -1)
    C1 = A1 @ V                   # [B,L,D]
    E2 = C1 @ K^T * SCALE
    A2 = softmax(E2, -1)
    C2 = A2 @ V
    return (C2, A2)

Sharding: 8 cores = (batch b in 0..3) x (label half h in 0..1). Each core
computes K/V for its batch and a 4096-label slice of the two attention passes.

Core-local algorithm (all matmuls bf16 with fp32 PSUM accumulation):
  - K kept transposed:  kt[d, s]   (4 tiles [128, 2048])
  - V kept natural:     v[s, d]    (16 tiles [128, 512])
  - bias folded into the conv matmuls via an augmented contraction row
    (H_aug row 512 = ones; W_aug row 512 = bias).
  - Pass 1 is computed entirely in transposed layout E1t[s, l]; softmax
    max-subtraction is skipped (logits are O(0.3), mathematically identical),
    row sums are obtained with a ones-vector matmul, and the normalization
    1/sum1 is folded into pass-2's exp scale (softmax is scale-invariant
    per row).
  - Pass 2 E2[l, s] is computed in natural layout for the A output and
    softmax stats; exp2 (unnormalized, bf16) is PE-transposed for the C2
    matmul, and C2 rows are scaled by 1/sum2 on the PSUM->SBUF copy.
"""

import contextlib

import numpy as np
import ml_dtypes

import bass_rust
import concourse.bass as bass
import concourse.mybir as mybir
import concourse.tile as tile
from concourse.bass_utils import run_bass_kernel_spmd
from concourse.masks import make_identity

B, D, S, L = 4, 512, 2048, 8192
N_CORES = 8
LH = L // 2  # labels per core
SCALE = 1.0 / float(np.sqrt(512.0))

F32 = mybir.dt.float32
BF16 = mybir.dt.bfloat16
AF = mybir.ActivationFunctionType
ALU = mybir.AluOpType

DC = D // 128  # 4 d-chunks
NS = S // 512  # 4 s-blocks of 512
TS = S // 128  # 16 s-tiles of 128
NG = LH // 512  # 8 label groups of 512
NT = 4  # l-tiles of 128 per group


def _split_multi_waits(nc, cap=1):
    """walrus in this container accepts at most one sem-wait per instruction;
    move extra waits onto preceding NOPs on the same engine."""
    for f in nc.m.functions:
        for blk in f.blocks:
            newlist = []
            changed = False
            for ins in blk.instructions:
                si = ins.sync_info
                if si is not None and len(si.on_wait) > cap:
                    waits = list(si.on_wait)
                    extra, keep = waits[:-cap], waits[-cap:]
                    for i in range(0, len(extra), cap):
                        nop = mybir.InstNoOp(
                            name=f"{ins.name}-wsplit{i}",
                            engine=ins.engine,
                            ins=[],
                            outs=[],
                            sync_info=bass_rust.SyncInfo(
                                on_wait=extra[i : i + cap], on_update=[]
                            ),
                        )
                        newlist.append(nop)
                    ins.sync_info = bass_rust.SyncInfo(
                        on_wait=keep, on_update=list(si.on_update)
                    )
                    changed = True
                newlist.append(ins)
            if changed:
                blk.instructions = newlist


def _emit(nc):
    Haug = nc.dram_tensor("Haug", [D + 1, S], BF16, kind="ExternalInput").ap()
    KWaug = nc.dram_tensor("KWaug", [D + 1, D], BF16, kind="ExternalInput").ap()
    VWaug = nc.dram_tensor("VWaug", [D + 1, D], BF16, kind="ExternalInput").ap()
    QT = nc.dram_tensor("QT", [D, LH], BF16, kind="ExternalInput").ap()
    C_out = nc.dram_tensor("C", [LH, D], F32, kind="ExternalOutput").ap()
    A_out = nc.dram_tensor("A", [LH, S], F32, kind="ExternalOutput").ap()

    mm = nc.tensor.matmul

    with tile.TileContext(nc) as tc, contextlib.ExitStack() as ctx:
        const = ctx.enter_context(tc.tile_pool(name="const", bufs=1))
        loads = ctx.enter_context(tc.tile_pool(name="loads", bufs=1))
        persist = ctx.enter_context(tc.tile_pool(name="persist", bufs=1))
        work = ctx.enter_context(tc.tile_pool(name="work", bufs=2))
        psum = ctx.enter_context(tc.tile_pool(name="psum", bufs=2, space="PSUM"))
        dram = ctx.enter_context(tc.tile_pool(name="dram", bufs=2, space="DRAM"))

        # ---- constants ----
        identity = const.tile([128, 128], BF16, name="identity")
        make_identity(nc, identity)
        ones_col = const.tile([128, 1], BF16, name="ones_col")
        nc.vector.memset(ones_col, 1.0)

        # ---- input loads ----
        h_sb = []
        for j in range(DC):
            h_j = loads.tile([128, S], BF16, name=f"h_{j}", tag=f"h{j}")
            nc.sync.dma_start(out=h_j, in_=Haug[128 * j : 128 * (j + 1), :])
            h_sb.append(h_j)
        hones = loads.tile([1, S], BF16, name="hones")
        nc.sync.dma_start(out=hones, in_=Haug[D : D + 1, :])

        kw_sb, vw_sb = [], []
        for j in range(DC):
            kw_j = loads.tile([128, D], BF16, name=f"kw_{j}", tag=f"kw{j}")
            nc.sync.dma_start(out=kw_j, in_=KWaug[128 * j : 128 * (j + 1), :])
            kw_sb.append(kw_j)
            vw_j = loads.tile([128, D], BF16, name=f"vw_{j}", tag=f"vw{j}")
            nc.sync.dma_start(out=vw_j, in_=VWaug[128 * j : 128 * (j + 1), :])
            vw_sb.append(vw_j)
        kwb = loads.tile([1, D], BF16, name="kwb")
        nc.sync.dma_start(out=kwb, in_=KWaug[D : D + 1, :])
        vwb = loads.tile([1, D], BF16, name="vwb")
        nc.sync.dma_start(out=vwb, in_=VWaug[D : D + 1, :])

        qt = []
        for j in range(DC):
            qt_j = persist.tile([128, LH], BF16, name=f"qt_{j}", tag=f"qt{j}")
            nc.sync.dma_start(out=qt_j, in_=QT[128 * j : 128 * (j + 1), :])
            qt.append(qt_j)

        # ---- ELU helper: out_bf16 = elu(psum) = relu(x) + exp(min(x,0)) - 1 ----
        def elu(ps, out_slice, key):
            r = work.tile([128, 512], F32, name=f"elu_r_{key}", tag="elu_r", bufs=3)
            nc.scalar.activation(r, ps, AF.Relu)
            mn = work.tile([128, 512], F32, name=f"elu_m_{key}", tag="elu_m", bufs=3)
            nc.vector.tensor_scalar_min(mn, ps, 0.0)
            e = work.tile([128, 512], F32, name=f"elu_e_{key}", tag="elu_e", bufs=3)
            nc.scalar.activation(e, mn, AF.Exp)
            nc.vector.scalar_tensor_tensor(
                out_slice, in0=e, scalar=-1.0, in1=r, op0=ALU.add, op1=ALU.add
            )

        # ---- stage 0: K (transposed layout) ----
        kt = []
        for m in range(DC):
            kt_m = persist.tile([128, S], BF16, name=f"kt_{m}", tag=f"kt{m}")
            for n in range(NS):
                ps = psum.tile([128, 512], F32, name=f"kc_{m}_{n}", tag="mm512")
                for j in range(DC):
                    mm(
                        ps,
                        lhsT=kw_sb[j][:, 128 * m : 128 * (m + 1)],
                        rhs=h_sb[j][:, 512 * n : 512 * (n + 1)],
                        start=(j == 0),
                        stop=False,
                    )
                mm(
                    ps,
                    lhsT=kwb[:, 128 * m : 128 * (m + 1)],
                    rhs=hones[:, 512 * n : 512 * (n + 1)],
                    start=False,
                    stop=True,
                )
                elu(ps, kt_m[:, 512 * n : 512 * (n + 1)], f"k{m}{n}")
            kt.append(kt_m)

        # ---- stage 0: V (natural layout) ----
        v = []
        for i in range(TS):
            v_i = persist.tile([128, D], BF16, name=f"v_{i}", tag=f"v{i}")
            ps = psum.tile([128, 512], F32, name=f"vc_{i}", tag="mm512")
            for j in range(DC):
                mm(
                    ps,
                    lhsT=h_sb[j][:, 128 * i : 128 * (i + 1)],
                    rhs=vw_sb[j],
                    start=(j == 0),
                    stop=False,
                )
            mm(
                ps,
                lhsT=hones[:, 128 * i : 128 * (i + 1)],
                rhs=vwb,
                start=False,
                stop=True,
            )
            elu(ps, v_i, f"v{i}")
            v.append(v_i)

        # ---- main loop over label groups of 512 ----
        for g in range(NG):
            # pass 1: E1t[s, l] = kt^T q, exp, row sums via ones-matmul
            sums_ps = psum.tile([1, 512], F32, name=f"sums_{g}", tag="sums", bufs=1)
            e1t = []
            for i in range(TS):
                ps = psum.tile([128, 512], F32, name=f"e1_{g}_{i}", tag="mm512")
                for j in range(DC):
                    mm(
                        ps,
                        lhsT=kt[j][:, 128 * i : 128 * (i + 1)],
                        rhs=qt[j][:, 512 * g : 512 * (g + 1)],
                        start=(j == 0),
                        stop=(j == DC - 1),
                    )
                e1t_i = work.tile(
                    [128, 512], BF16, name=f"e1t_{g}_{i}", tag=f"e1t{i}", bufs=2
                )
                nc.scalar.activation(e1t_i, ps, AF.Exp, scale=SCALE)
                mm(
                    sums_ps,
                    lhsT=ones_col,
                    rhs=e1t_i,
                    start=(i == 0),
                    stop=(i == TS - 1),
                )
                e1t.append(e1t_i)

            # 1/sum1 path: psum[1,512] -> sbuf -> dram -> sbuf[128,4] -> recip
            sums_sb = work.tile([1, 512], F32, name=f"sums_sb_{g}", tag="sums_sb")
            nc.vector.tensor_copy(sums_sb, sums_ps)
            dscr = dram.tile([512], F32, name=f"dscr_{g}", tag="dscr")
            nc.sync.dma_start(out=dscr, in_=sums_sb)
            rsum = work.tile([128, 4], F32, name=f"rsum_{g}", tag="rsum")
            nc.sync.dma_start(out=rsum, in_=dscr.rearrange("(f p) -> p f", p=128))
            s2 = work.tile([128, 4], F32, name=f"s2_{g}", tag="s2")
            nc.vector.reciprocal(s2, rsum)
            nc.vector.tensor_scalar_mul(s2, s2, SCALE)

            # C1t[d, l] = sum_s v^T exp1t  (unnormalized)
            c1t = []
            for j in range(DC):
                ps = psum.tile([128, 512], F32, name=f"c1_{g}_{j}", tag="mm512")
                for i in range(TS):
                    mm(
                        ps,
                        lhsT=v[i][:, 128 * j : 128 * (j + 1)],
                        rhs=e1t[i],
                        start=(i == 0),
                        stop=(i == TS - 1),
                    )
                c1t_j = work.tile(
                    [128, 512], BF16, name=f"c1t_{g}_{j}", tag=f"c1t{j}", bufs=2
                )
                nc.vector.tensor_copy(c1t_j, ps)
                c1t.append(c1t_j)

            # pass 2 per l-tile of 128
            for t in range(NT):
                row0 = 512 * g + 128 * t
                s2_t = s2[:, t : t + 1]
                # E2[l, s] in two PSUM halves
                e2h = []
                for hlf in range(2):
                    ps = psum.tile(
                        [128, 1024], F32, name=f"e2_{g}_{t}_{hlf}", tag="e2h"
                    )
                    for nn in range(2):
                        n = 2 * hlf + nn
                        for j in range(DC):
                            mm(
                                ps[:, 512 * nn : 512 * (nn + 1)],
                                lhsT=c1t[j][:, 128 * t : 128 * (t + 1)],
                                rhs=kt[j][:, 512 * n : 512 * (n + 1)],
                                start=(j == 0),
                                stop=(j == DC - 1),
                            )
                    e2h.append(ps)

                nmax_a = work.tile([128, 1], F32, name=f"nma_{g}_{t}", tag="nma")
                nc.vector.reduce_max(
                    nmax_a, e2h[0], axis=mybir.AxisListType.X, negate=True
                )
                nmax_b = work.tile([128, 1], F32, name=f"nmb_{g}_{t}", tag="nmb")
                nc.vector.reduce_max(
                    nmax_b, e2h[1], axis=mybir.AxisListType.X, negate=True
                )
                bias2 = work.tile([128, 1], F32, name=f"bias2_{g}_{t}", tag="bias2")
                nc.vector.tensor_tensor(bias2, nmax_a, nmax_b, op=ALU.min)
                nc.vector.tensor_mul(bias2, bias2, s2_t)

                exp2 = work.tile([128, S], BF16, name=f"exp2_{g}_{t}", tag="exp2")
                sum2h = work.tile([128, 2], F32, name=f"sum2h_{g}_{t}", tag="sum2h")
                for hlf in range(2):
                    nc.scalar.activation(
                        exp2[:, 1024 * hlf : 1024 * (hlf + 1)],
                        e2h[hlf],
                        AF.Exp,
                        bias=bias2,
                        scale=s2_t,
                        accum_out=sum2h[:, hlf : hlf + 1],
                    )
                recip2 = work.tile([128, 1], F32, name=f"recip2_{g}_{t}", tag="recip2")
                nc.vector.tensor_add(recip2, sum2h[:, 0:1], sum2h[:, 1:2])
                nc.vector.reciprocal(recip2, recip2)

                # normalized A2 out (fp32)
                a2 = work.tile([128, S], F32, name=f"a2_{g}_{t}", tag="a2")
                nc.vector.tensor_scalar_mul(a2, exp2, recip2)
                nc.sync.dma_start(out=A_out[row0 : row0 + 128, :], in_=a2)

                # C2: transpose exp2 in batches of 4, then one accumulation chain
                a2t = []
                for q in range(4):
                    tp = psum.tile([128, 512], BF16, name=f"tp_{g}_{t}_{q}", tag="mm512")
                    for r in range(4):
                        i = 4 * q + r
                        nc.tensor.transpose(
                            tp[:, 128 * r : 128 * (r + 1)],
                            exp2[:, 128 * i : 128 * (i + 1)],
                            identity,
                        )
                    a2t_q = work.tile(
                        [128, 512], BF16, name=f"a2t_{g}_{t}_{q}", tag="a2t", bufs=5
                    )
                    nc.vector.tensor_copy(a2t_q, tp)
                    a2t.append(a2t_q)

                c2ps = psum.tile([128, 512], F32, name=f"c2_{g}_{t}", tag="c2", bufs=1)
                for i in range(TS):
                    q, r = divmod(i, 4)
                    mm(
                        c2ps,
                        lhsT=a2t[q][:, 128 * r : 128 * (r + 1)],
                        rhs=v[i],
                        start=(i == 0),
                        stop=(i == TS - 1),
                    )
                c2sb = work.tile([128, D], F32, name=f"c2sb_{g}_{t}", tag="c2sb")
                nc.scalar.activation(c2sb, c2ps, AF.Copy, scale=recip2)
                nc.sync.dma_start(out=C_out[row0 : row0 + 128, :], in_=c2sb)


_CACHE = {}


def _build(split_waits=True):
    key = ("nc", split_waits)
    if key not in _CACHE:
        nc = bass.Bass(
            "TRN2", target_bir_lowering=False, debug=False, num_devices=N_CORES
        )
        _emit(nc)
        if split_waits:
            _split_multi_waits(nc)
        _CACHE[key] = nc
    return _CACHE[key]


def kernel(H, K_w, K_b, V_w, V_b, Q_w):
    H = np.asarray(H, dtype=np.float32)
    K_w = np.asarray(K_w, dtype=np.float32)
    K_b = np.asarray(K_b, dtype=np.float32)
    V_w = np.asarray(V_w, dtype=np.float32)
    V_b = np.asarray(V_b, dtype=np.float32)
    Q_w = np.asarray(Q_w, dtype=np.float32)

    bf = ml_dtypes.bfloat16
    kw_aug = np.concatenate([K_w.T, K_b[None, :]], axis=0).astype(bf)  # [513, 512]
    vw_aug = np.concatenate([V_w.T, V_b[None, :]], axis=0).astype(bf)
    ones_row = np.ones((1, S), dtype=np.float32)

    in_maps = []
    for c in range(N_CORES):
        b, h = divmod(c, 2)
        h_aug = np.concatenate([H[b], ones_row], axis=0).astype(bf)  # [513, 2048]
        q_t = np.ascontiguousarray(Q_w[h * LH : (h + 1) * LH, :].T).astype(bf)
        in_maps.append(
            {"Haug": h_aug, "KWaug": kw_aug, "VWaug": vw_aug, "QT": q_t}
        )

    nc = _build()
    res = run_bass_kernel_spmd(nc, in_maps, core_ids=list(range(N_CORES)))

    C_full = np.empty((B, L, D), dtype=np.float32)
    A_full = np.empty((B, L, S), dtype=np.float32)
    for c in range(N_CORES):
        b, h = divmod(c, 2)
        C_full[b, h * LH : (h + 1) * LH, :] = res.results[c]["C"]
        A_full[b, h * LH : (h + 1) * LH, :] = res.results[c]["A"]
    return C_full, A_full


# revision 8
# speedup vs baseline: 1.0575x; 1.0575x over previous
"""Trainium2 Bass kernel for nn_ContextAttention (two-pass label attention).

Reference computation (fp32):
    K = elu(K_w @ H + K_b)        # [B,S,D] keys,  per-position linear over channels
    V = elu(V_w @ H + V_b)        # [B,S,D]
    E1 = Q_w @ K^T * SCALE        # [B,L,S]
    A1 = softmax(E1, -1)
    C1 = A1 @ V                   # [B,L,D]
    E2 = C1 @ K^T * SCALE
    A2 = softmax(E2, -1)
    C2 = A2 @ V
    return (C2, A2)

Sharding: 8 cores = (batch b in 0..3) x (label half h in 0..1). Each core
computes K/V for its batch and a 4096-label slice of the two attention passes.

Core-local algorithm (all matmuls bf16 with fp32 PSUM accumulation):
  - K kept transposed:  kt[d, s]   (4 tiles [128, 2048]); K bias applied on
    the per-partition path of the ELU (bias == partition dim there).
  - V kept natural:     v[s, d]    (16 tiles [128, 512]); V bias folded into
    the conv matmul via an augmented contraction row (H_aug row 512 = ones,
    VW_aug row 512 = bias) since it lands on the free dim.
  - Pass 1 is computed entirely in transposed layout E1t[s, l]; softmax
    max-subtraction is skipped (logits are O(0.3), mathematically identical),
    row sums are computed on GPSIMD (partition reduce), and the 1/sum1
    normalization is folded into pass-2's exp scale (softmax is row-scale
    invariant).
  - Pass 2 E2[l, s] is computed in natural layout for the A output and
    softmax stats; exp2 (unnormalized, bf16) is transposed with one DMA
    XBAR-transpose per l-tile for the C2 matmul, and C2 rows are scaled by
    1/sum2 on the PSUM->SBUF copy.
"""

import contextlib

import numpy as np
import ml_dtypes

import bass_rust
import concourse.bass as bass
import concourse.mybir as mybir
import concourse.tile as tile
from concourse.bass_utils import run_bass_kernel_spmd
from concourse.masks import make_identity

B, D, S, L = 4, 512, 2048, 8192
N_CORES = 8
LH = L // 2  # labels per core
SCALE = 1.0 / float(np.sqrt(512.0))

F32 = mybir.dt.float32
BF16 = mybir.dt.bfloat16
AF = mybir.ActivationFunctionType
ALU = mybir.AluOpType

DC = D // 128  # 4 d-chunks
NS = S // 512  # 4 s-blocks of 512
TS = S // 128  # 16 s-tiles of 128
NG = LH // 512  # 8 label groups of 512
NT = 4  # l-tiles of 128 per group


def _split_multi_waits(nc, cap=1):
    """walrus in this container accepts at most one sem-wait per instruction;
    move extra waits onto preceding NOPs on the same engine."""
    for f in nc.m.functions:
        for blk in f.blocks:
            newlist = []
            changed = False
            for ins in blk.instructions:
                si = ins.sync_info
                if si is not None and len(si.on_wait) > cap:
                    waits = list(si.on_wait)
                    extra, keep = waits[:-cap], waits[-cap:]
                    for i in range(0, len(extra), cap):
                        nop = mybir.InstNoOp(
                            name=f"{ins.name}-wsplit{i}",
                            engine=ins.engine,
                            ins=[],
                            outs=[],
                            sync_info=bass_rust.SyncInfo(
                                on_wait=extra[i : i + cap], on_update=[]
                            ),
                        )
                        newlist.append(nop)
                    ins.sync_info = bass_rust.SyncInfo(
                        on_wait=keep, on_update=list(si.on_update)
                    )
                    changed = True
                newlist.append(ins)
            if changed:
                blk.instructions = newlist


def _emit(nc):
    Haug = nc.dram_tensor("Haug", [D + 1, S], BF16, kind="ExternalInput").ap()
    KWT = nc.dram_tensor("KWT", [D, D], BF16, kind="ExternalInput").ap()
    Kb = nc.dram_tensor("Kb", [D], F32, kind="ExternalInput").ap()
    VWaug = nc.dram_tensor("VWaug", [D + 1, D], BF16, kind="ExternalInput").ap()
    QT = nc.dram_tensor("QT", [D, LH], BF16, kind="ExternalInput").ap()
    C_out = nc.dram_tensor("C", [LH, D], F32, kind="ExternalOutput").ap()
    A_out = nc.dram_tensor("A", [LH, S], F32, kind="ExternalOutput").ap()

    mm = nc.tensor.matmul
    import concourse.bass_isa as bass_isa

    with tile.TileContext(nc) as tc, contextlib.ExitStack() as ctx:
        const = ctx.enter_context(tc.tile_pool(name="const", bufs=1))
        loads = ctx.enter_context(tc.tile_pool(name="loads", bufs=1))
        persist = ctx.enter_context(tc.tile_pool(name="persist", bufs=1))
        work = ctx.enter_context(tc.tile_pool(name="work", bufs=2))
        psum = ctx.enter_context(tc.tile_pool(name="psum", bufs=3, space="PSUM"))
        dram = ctx.enter_context(tc.tile_pool(name="dram", bufs=2, space="DRAM"))

        # ---- input loads ----
        h_sb = []
        for j in range(DC):
            h_j = loads.tile([128, S], BF16, name=f"h_{j}", tag=f"h{j}")
            nc.sync.dma_start(out=h_j, in_=Haug[128 * j : 128 * (j + 1), :])
            h_sb.append(h_j)
        hones = loads.tile([1, S], BF16, name="hones")
        nc.sync.dma_start(out=hones, in_=Haug[D : D + 1, :])

        kw_sb, vw_sb = [], []
        for j in range(DC):
            kw_j = loads.tile([128, D], BF16, name=f"kw_{j}", tag=f"kw{j}")
            nc.sync.dma_start(out=kw_j, in_=KWT[128 * j : 128 * (j + 1), :])
            kw_sb.append(kw_j)
            vw_j = loads.tile([128, D], BF16, name=f"vw_{j}", tag=f"vw{j}")
            nc.sync.dma_start(out=vw_j, in_=VWaug[128 * j : 128 * (j + 1), :])
            vw_sb.append(vw_j)
        vwb = loads.tile([1, D], BF16, name="vwb")
        nc.sync.dma_start(out=vwb, in_=VWaug[D : D + 1, :])
        # K bias as [128, 4]: column m holds K_b[128m : 128(m+1)]
        kb_sb = loads.tile([128, DC], F32, name="kb_sb")
        nc.sync.dma_start(out=kb_sb, in_=Kb.rearrange("(c p) -> p c", p=128))

        qt = []
        for j in range(DC):
            qt_j = persist.tile([128, LH], BF16, name=f"qt_{j}", tag=f"qt{j}")
            nc.sync.dma_start(out=qt_j, in_=QT[128 * j : 128 * (j + 1), :])
            qt.append(qt_j)

        # ---- ELU helper: out_bf16 = elu(ps + bias) ----
        # relu(x) + exp(min(x,0)) - 1, bias optional per-partition [P,1] fp32
        def elu(ps, out_slice, key, bias_col=None):
            r = work.tile([128, 512], F32, name=f"elu_r_{key}", tag="elu_r", bufs=3)
            mn = work.tile([128, 512], F32, name=f"elu_m_{key}", tag="elu_m", bufs=3)
            if bias_col is None:
                nc.scalar.activation(r, ps, AF.Relu)
                nc.vector.tensor_scalar_min(mn, ps, 0.0)
            else:
                nc.scalar.activation(r, ps, AF.Relu, bias=bias_col)
                nc.vector.tensor_scalar(
                    mn, ps, scalar1=bias_col, scalar2=0.0, op0=ALU.add, op1=ALU.min
                )
            e = work.tile([128, 512], F32, name=f"elu_e_{key}", tag="elu_e", bufs=3)
            nc.scalar.activation(e, mn, AF.Exp)
            nc.vector.scalar_tensor_tensor(
                out_slice, in0=e, scalar=-1.0, in1=r, op0=ALU.add, op1=ALU.add
            )

        # ---- stage 0: K (transposed layout, per-partition bias) ----
        kt = []
        for m in range(DC):
            kt_m = persist.tile([128, S], BF16, name=f"kt_{m}", tag=f"kt{m}")
            for n in range(NS):
                ps = psum.tile([128, 512], F32, name=f"kc_{m}_{n}", tag="mm512")
                for j in range(DC):
                    mm(
                        ps,
                        lhsT=kw_sb[j][:, 128 * m : 128 * (m + 1)],
                        rhs=h_sb[j][:, 512 * n : 512 * (n + 1)],
                        start=(j == 0),
                        stop=(j == DC - 1),
                    )
                elu(ps, kt_m[:, 512 * n : 512 * (n + 1)], f"k{m}{n}",
                    bias_col=kb_sb[:, m : m + 1])
            kt.append(kt_m)

        # ---- stage 0: V (natural layout, bias via augmented row) ----
        v = []
        for i in range(TS):
            v_i = persist.tile([128, D], BF16, name=f"v_{i}", tag=f"v{i}")
            ps = psum.tile([128, 512], F32, name=f"vc_{i}", tag="mm512")
            for j in range(DC):
                mm(
                    ps,
                    lhsT=h_sb[j][:, 128 * i : 128 * (i + 1)],
                    rhs=vw_sb[j],
                    start=(j == 0),
                    stop=False,
                )
            mm(
                ps,
                lhsT=hones[:, 128 * i : 128 * (i + 1)],
                rhs=vwb,
                start=False,
                stop=True,
            )
            elu(ps, v_i, f"v{i}")
            v.append(v_i)

        ones_col = const.tile([128, 1], BF16, name="ones_col")
        nc.vector.memset(ones_col, 1.0)

        # ---- main loop over label groups of 512 ----
        for g in range(NG):
            # pass 1: E1t[s, l] = kt^T q -> exp; row sums via ones-matmul.
            # sums shares the "c2" psum tag: c2 is only live in pass 2, sums
            # only in pass 1, so the two phases share the same two banks.
            sums_ps = psum.tile([1, 512], F32, name=f"sums_{g}", tag="c2", bufs=1)
            e1t = []
            for i in range(TS):
                ps = psum.tile([128, 512], F32, name=f"e1_{g}_{i}", tag="mm512")
                for j in range(DC):
                    mm(
                        ps,
                        lhsT=kt[j][:, 128 * i : 128 * (i + 1)],
                        rhs=qt[j][:, 512 * g : 512 * (g + 1)],
                        start=(j == 0),
                        stop=(j == DC - 1),
                    )
                e1t_i = work.tile(
                    [128, 512], BF16, name=f"e1t_{g}_{i}", tag=f"e1t{i}", bufs=2
                )
                nc.scalar.activation(e1t_i, ps, AF.Exp, scale=SCALE)
                mm(
                    sums_ps,
                    lhsT=ones_col,
                    rhs=e1t_i,
                    start=(i == 0),
                    stop=(i == TS - 1),
                )
                e1t.append(e1t_i)

            # 1/sum1 path: psum[1,512] -> sbuf -> dram -> sbuf[128,4] -> recip
            sums_sb = work.tile([1, 512], F32, name=f"sums_sb_{g}", tag="sums_sb")
            nc.vector.tensor_copy(sums_sb, sums_ps)
            dscr = dram.tile([512], F32, name=f"dscr_{g}", tag="dscr")
            nc.sync.dma_start(out=dscr, in_=sums_sb)
            rsum = work.tile([128, 4], F32, name=f"rsum_{g}", tag="rsum")
            nc.sync.dma_start(out=rsum, in_=dscr.rearrange("(f p) -> p f", p=128))
            s2 = work.tile([128, 4], F32, name=f"s2_{g}", tag="s2")
            nc.vector.reciprocal(s2, rsum)
            nc.vector.tensor_scalar_mul(s2, s2, SCALE)

            # C1t[d, l] = sum_s v^T exp1t  (unnormalized)
            c1t = []
            for j in range(DC):
                ps = psum.tile([128, 512], F32, name=f"c1_{g}_{j}", tag="mm512")
                for i in range(TS):
                    mm(
                        ps,
                        lhsT=v[i][:, 128 * j : 128 * (j + 1)],
                        rhs=e1t[i],
                        start=(i == 0),
                        stop=(i == TS - 1),
                    )
                c1t_j = work.tile(
                    [128, 512], BF16, name=f"c1t_{g}_{j}", tag=f"c1t{j}", bufs=2
                )
                nc.vector.tensor_copy(c1t_j, ps)
                c1t.append(c1t_j)

            # pass 2 per l-tile of 128
            for t in range(NT):
                row0 = 512 * g + 128 * t
                s2_t = s2[:, t : t + 1]
                # E2[l, s] in four 1-bank PSUM quarters (finer rotation)
                e2q = []
                for n in range(NS):
                    ps = psum.tile(
                        [128, 512], F32, name=f"e2_{g}_{t}_{n}", tag="e2q", bufs=4
                    )
                    for j in range(DC):
                        mm(
                            ps,
                            lhsT=c1t[j][:, 128 * t : 128 * (t + 1)],
                            rhs=kt[j][:, 512 * n : 512 * (n + 1)],
                            start=(j == 0),
                            stop=(j == DC - 1),
                        )
                    e2q.append(ps)

                nmax = work.tile([128, NS], F32, name=f"nmax_{g}_{t}", tag="nmax")
                for n in range(NS):
                    nc.vector.reduce_max(
                        nmax[:, n : n + 1], e2q[n], axis=mybir.AxisListType.X,
                        negate=True,
                    )
                bias2 = work.tile([128, 1], F32, name=f"bias2_{g}_{t}", tag="bias2")
                nc.vector.tensor_reduce(
                    bias2, nmax, axis=mybir.AxisListType.X, op=ALU.min
                )
                nc.vector.tensor_mul(bias2, bias2, s2_t)

                exp2 = work.tile([128, S], BF16, name=f"exp2_{g}_{t}", tag="exp2")
                sum2h = work.tile([128, NS], F32, name=f"sum2h_{g}_{t}", tag="sum2h")
                for n in range(NS):
                    nc.scalar.activation(
                        exp2[:, 512 * n : 512 * (n + 1)],
                        e2q[n],
                        AF.Exp,
                        bias=bias2,
                        scale=s2_t,
                        accum_out=sum2h[:, n : n + 1],
                    )
                recip2 = work.tile([128, 1], F32, name=f"recip2_{g}_{t}", tag="recip2")
                nc.vector.tensor_reduce(
                    recip2, sum2h, axis=mybir.AxisListType.X, op=ALU.add
                )
                nc.vector.reciprocal(recip2, recip2)

                # normalized A2 out (fp32)
                a2 = work.tile([128, S], F32, name=f"a2_{g}_{t}", tag="a2")
                nc.vector.tensor_scalar_mul(a2, exp2, recip2)
                nc.sync.dma_start(out=A_out[row0 : row0 + 128, :], in_=a2)

                # C2: one XBAR DMA transpose of exp2 -> [s, l] blocks
                a2t = work.tile(
                    [128, TS, 128], BF16, name=f"a2t_{g}_{t}", tag="a2t", bufs=2
                )
                nc.sync.dma_start(out=a2t, in_=exp2, transpose=True)

                c2ps = psum.tile([128, 512], F32, name=f"c2_{g}_{t}", tag="c2", bufs=1)
                for i in range(TS):
                    mm(
                        c2ps,
                        lhsT=a2t[:, i, :],
                        rhs=v[i],
                        start=(i == 0),
                        stop=(i == TS - 1),
                    )
                c2sb = work.tile([128, D], F32, name=f"c2sb_{g}_{t}", tag="c2sb")
                nc.scalar.activation(c2sb, c2ps, AF.Copy, scale=recip2)
                nc.sync.dma_start(out=C_out[row0 : row0 + 128, :], in_=c2sb)


_CACHE = {}


def _build(split_waits=True):
    key = ("nc", split_waits)
    if key not in _CACHE:
        nc = bass.Bass(
            "TRN2", target_bir_lowering=False, debug=False, num_devices=N_CORES
        )
        _emit(nc)
        if split_waits:
            _split_multi_waits(nc)
        _CACHE[key] = nc
    return _CACHE[key]


def kernel(H, K_w, K_b, V_w, V_b, Q_w):
    H = np.asarray(H, dtype=np.float32)
    K_w = np.asarray(K_w, dtype=np.float32)
    K_b = np.asarray(K_b, dtype=np.float32)
    V_w = np.asarray(V_w, dtype=np.float32)
    V_b = np.asarray(V_b, dtype=np.float32)
    Q_w = np.asarray(Q_w, dtype=np.float32)

    bf = ml_dtypes.bfloat16
    kwt = np.ascontiguousarray(K_w.T).astype(bf)  # [512, 512]
    vw_aug = np.concatenate([V_w.T, V_b[None, :]], axis=0).astype(bf)  # [513, 512]
    ones_row = np.ones((1, S), dtype=np.float32)

    in_maps = []
    for c in range(N_CORES):
        b, h = divmod(c, 2)
        h_aug = np.concatenate([H[b], ones_row], axis=0).astype(bf)  # [513, 2048]
        q_t = np.ascontiguousarray(Q_w[h * LH : (h + 1) * LH, :].T).astype(bf)
        in_maps.append(
            {"Haug": h_aug, "KWT": kwt, "Kb": K_b, "VWaug": vw_aug, "QT": q_t}
        )

    nc = _build()
    res = run_bass_kernel_spmd(nc, in_maps, core_ids=list(range(N_CORES)))

    C_full = np.empty((B, L, D), dtype=np.float32)
    A_full = np.empty((B, L, S), dtype=np.float32)
    for c in range(N_CORES):
        b, h = divmod(c, 2)
        C_full[b, h * LH : (h + 1) * LH, :] = res.results[c]["C"]
        A_full[b, h * LH : (h + 1) * LH, :] = res.results[c]["A"]
    return C_full, A_full


# revision 9
# speedup vs baseline: 20390.9451x; 19282.6878x over previous
"""Trainium2 Bass kernel for nn_ContextAttention (two-pass label attention).

Reference computation (fp32):
    K = elu(K_w @ H + K_b)        # [B,S,D] keys,  per-position linear over channels
    V = elu(V_w @ H + V_b)        # [B,S,D]
    E1 = Q_w @ K^T * SCALE        # [B,L,S]
    A1 = softmax(E1, -1)
    C1 = A1 @ V                   # [B,L,D]
    E2 = C1 @ K^T * SCALE
    A2 = softmax(E2, -1)
    C2 = A2 @ V
    return (C2, A2)

Sharding: 8 cores = (batch b in 0..3) x (label half h in 0..1). Each core
computes K/V for its batch and a 4096-label slice of the two attention passes.

Core-local algorithm (all matmuls bf16 with fp32 PSUM accumulation):
  - K kept transposed:  kt[d, s]   (4 tiles [128, 2048]); K bias applied on
    the per-partition path of the ELU (bias == partition dim there).
  - V kept natural:     v[s, d]    (16 tiles [128, 512]); V bias folded into
    the conv matmul via an augmented contraction row (H_aug row 512 = ones,
    VW_aug row 512 = bias) since it lands on the free dim.
  - Pass 1 is computed entirely in transposed layout E1t[s, l]; softmax
    max-subtraction is skipped (logits are O(0.3), mathematically identical),
    row sums are computed with a ones-vector matmul, and the 1/sum1
    normalization is folded into pass-2's exp scale (softmax is row-scale
    invariant).
  - Pass 2 E2[l, s] is computed in natural layout for the A output and
    softmax stats; exp2 (unnormalized, bf16) is transposed with one DMA
    XBAR-transpose per l-tile for the C2 matmul, and C2 rows are scaled by
    1/sum2 on the PSUM->SBUF copy.
"""

import contextlib

import numpy as np
import ml_dtypes

import bass_rust
import concourse.bass as bass
import concourse.mybir as mybir
import concourse.tile as tile
from concourse.bass_utils import run_bass_kernel_spmd

B, D, S, L = 4, 512, 2048, 8192
N_CORES = 8
LH = L // 2  # labels per core
SCALE = 1.0 / float(np.sqrt(512.0))

F32 = mybir.dt.float32
BF16 = mybir.dt.bfloat16
AF = mybir.ActivationFunctionType
ALU = mybir.AluOpType

DC = D // 128  # 4 d-chunks
NS = S // 512  # 4 s-blocks of 512
TS = S // 128  # 16 s-tiles of 128
NG = LH // 512  # 8 label groups of 512
NT = 4  # l-tiles of 128 per group


def _split_multi_waits(nc, cap=1):
    """walrus in this container accepts at most one sem-wait per instruction;
    move extra waits onto preceding NOPs on the same engine."""
    for f in nc.m.functions:
        for blk in f.blocks:
            newlist = []
            changed = False
            for ins in blk.instructions:
                si = ins.sync_info
                if si is not None and len(si.on_wait) > cap:
                    waits = list(si.on_wait)
                    extra, keep = waits[:-cap], waits[-cap:]
                    for i in range(0, len(extra), cap):
                        nop = mybir.InstNoOp(
                            name=f"{ins.name}-wsplit{i}",
                            engine=ins.engine,
                            ins=[],
                            outs=[],
                            sync_info=bass_rust.SyncInfo(
                                on_wait=extra[i : i + cap], on_update=[]
                            ),
                        )
                        newlist.append(nop)
                    ins.sync_info = bass_rust.SyncInfo(
                        on_wait=keep, on_update=list(si.on_update)
                    )
                    changed = True
                newlist.append(ins)
            if changed:
                blk.instructions = newlist


def _emit(nc):
    Haug = nc.dram_tensor("Haug", [D + 1, S], BF16, kind="ExternalInput").ap()
    KWT = nc.dram_tensor("KWT", [D, D], BF16, kind="ExternalInput").ap()
    Kb = nc.dram_tensor("Kb", [D], F32, kind="ExternalInput").ap()
    VWaug = nc.dram_tensor("VWaug", [D + 1, D], BF16, kind="ExternalInput").ap()
    QT = nc.dram_tensor("QT", [D, LH], BF16, kind="ExternalInput").ap()
    C_out = nc.dram_tensor("C", [LH, D], F32, kind="ExternalOutput").ap()
    A_out = nc.dram_tensor("A", [LH, S], F32, kind="ExternalOutput").ap()

    mm = nc.tensor.matmul

    with tile.TileContext(nc) as tc, contextlib.ExitStack() as ctx:
        const = ctx.enter_context(tc.tile_pool(name="const", bufs=1))
        loads = ctx.enter_context(tc.tile_pool(name="loads", bufs=1))
        persist = ctx.enter_context(tc.tile_pool(name="persist", bufs=1))
        work = ctx.enter_context(tc.tile_pool(name="work", bufs=2))
        psum = ctx.enter_context(tc.tile_pool(name="psum", bufs=3, space="PSUM"))
        dram = ctx.enter_context(tc.tile_pool(name="dram", bufs=2, space="DRAM"))

        # ---- input loads ----
        h_sb = []
        for j in range(DC):
            h_j = loads.tile([128, S], BF16, name=f"h_{j}", tag=f"h{j}")
            nc.sync.dma_start(out=h_j, in_=Haug[128 * j : 128 * (j + 1), :])
            h_sb.append(h_j)
        hones = loads.tile([1, S], BF16, name="hones")
        nc.sync.dma_start(out=hones, in_=Haug[D : D + 1, :])

        kw_sb, vw_sb = [], []
        for j in range(DC):
            kw_j = loads.tile([128, D], BF16, name=f"kw_{j}", tag=f"kw{j}")
            nc.sync.dma_start(out=kw_j, in_=KWT[128 * j : 128 * (j + 1), :])
            kw_sb.append(kw_j)
            vw_j = loads.tile([128, D], BF16, name=f"vw_{j}", tag=f"vw{j}")
            nc.sync.dma_start(out=vw_j, in_=VWaug[128 * j : 128 * (j + 1), :])
            vw_sb.append(vw_j)
        vwb = loads.tile([1, D], BF16, name="vwb")
        nc.sync.dma_start(out=vwb, in_=VWaug[D : D + 1, :])
        # K bias as [128, 4]: column m holds K_b[128m : 128(m+1)]
        kb_sb = loads.tile([128, DC], F32, name="kb_sb")
        nc.sync.dma_start(out=kb_sb, in_=Kb.rearrange("(c p) -> p c", p=128))

        qt = []
        for j in range(DC):
            qt_j = persist.tile([128, LH], BF16, name=f"qt_{j}", tag=f"qt{j}")
            nc.sync.dma_start(out=qt_j, in_=QT[128 * j : 128 * (j + 1), :])
            qt.append(qt_j)

        # ---- ELU helper: out_bf16 = elu(ps + bias) ----
        # relu(x) + exp(min(x,0)) - 1, bias optional per-partition [P,1] fp32
        def elu(ps, out_slice, key, bias_col=None):
            r = work.tile([128, 512], F32, name=f"elu_r_{key}", tag="elu_r", bufs=3)
            mn = work.tile([128, 512], F32, name=f"elu_m_{key}", tag="elu_m", bufs=3)
            if bias_col is None:
                nc.scalar.activation(r, ps, AF.Relu)
                nc.vector.tensor_scalar_min(mn, ps, 0.0)
            else:
                nc.scalar.activation(r, ps, AF.Relu, bias=bias_col)
                nc.vector.tensor_scalar(
                    mn, ps, scalar1=bias_col, scalar2=0.0, op0=ALU.add, op1=ALU.min
                )
            e = work.tile([128, 512], F32, name=f"elu_e_{key}", tag="elu_e", bufs=3)
            nc.scalar.activation(e, mn, AF.Exp)
            nc.vector.scalar_tensor_tensor(
                out_slice, in0=e, scalar=-1.0, in1=r, op0=ALU.add, op1=ALU.add
            )

        # ---- stage 0: K (transposed layout, per-partition bias) ----
        kt = []
        for m in range(DC):
            kt_m = persist.tile([128, S], BF16, name=f"kt_{m}", tag=f"kt{m}")
            for n in range(NS):
                ps = psum.tile([128, 512], F32, name=f"kc_{m}_{n}", tag="mm512")
                for j in range(DC):
                    mm(
                        ps,
                        lhsT=kw_sb[j][:, 128 * m : 128 * (m + 1)],
                        rhs=h_sb[j][:, 512 * n : 512 * (n + 1)],
                        start=(j == 0),
                        stop=(j == DC - 1),
                    )
                elu(ps, kt_m[:, 512 * n : 512 * (n + 1)], f"k{m}{n}",
                    bias_col=kb_sb[:, m : m + 1])
            kt.append(kt_m)

        # ---- stage 0: V (natural layout, bias via augmented row) ----
        v = []
        for i in range(TS):
            v_i = persist.tile([128, D], BF16, name=f"v_{i}", tag=f"v{i}")
            ps = psum.tile([128, 512], F32, name=f"vc_{i}", tag="mm512")
            for j in range(DC):
                mm(
                    ps,
                    lhsT=h_sb[j][:, 128 * i : 128 * (i + 1)],
                    rhs=vw_sb[j],
                    start=(j == 0),
                    stop=False,
                )
            mm(
                ps,
                lhsT=hones[:, 128 * i : 128 * (i + 1)],
                rhs=vwb,
                start=False,
                stop=True,
            )
            elu(ps, v_i, f"v{i}")
            v.append(v_i)

        ones_col = const.tile([128, 1], BF16, name="ones_col")
        nc.vector.memset(ones_col, 1.0)

        # ---- main loop over label groups of 512 ----
        for g in range(NG):
            # pass 1: E1t[s, l] = kt^T q -> exp; row sums via ones-matmul.
            # sums shares the "c2" psum tag: c2 is only live in pass 2, sums
            # only in pass 1, so the two phases share the same two banks.
            sums_ps = psum.tile([1, 512], F32, name=f"sums_{g}", tag="c2", bufs=1)
            e1t = []
            for i in range(TS):
                ps = psum.tile([128, 512], F32, name=f"e1_{g}_{i}", tag="mm512")
                for j in range(DC):
                    mm(
                        ps,
                        lhsT=kt[j][:, 128 * i : 128 * (i + 1)],
                        rhs=qt[j][:, 512 * g : 512 * (g + 1)],
                        start=(j == 0),
                        stop=(j == DC - 1),
                    )
                e1t_i = work.tile(
                    [128, 512], BF16, name=f"e1t_{g}_{i}", tag=f"e1t{i}", bufs=2
                )
                nc.scalar.activation(e1t_i, ps, AF.Exp, scale=SCALE)
                mm(
                    sums_ps,
                    lhsT=ones_col,
                    rhs=e1t_i,
                    start=(i == 0),
                    stop=(i == TS - 1),
                )
                e1t.append(e1t_i)

            # 1/sum1 path: psum[1,512] -> sbuf -> dram -> sbuf[128,4] -> recip
            sums_sb = work.tile([1, 512], F32, name=f"sums_sb_{g}", tag="sums_sb")
            nc.vector.tensor_copy(sums_sb, sums_ps)
            dscr = dram.tile([512], F32, name=f"dscr_{g}", tag="dscr")
            nc.sync.dma_start(out=dscr, in_=sums_sb)
            rsum = work.tile([128, 4], F32, name=f"rsum_{g}", tag="rsum")
            nc.sync.dma_start(out=rsum, in_=dscr.rearrange("(f p) -> p f", p=128))
            s2 = work.tile([128, 4], F32, name=f"s2_{g}", tag="s2")
            nc.vector.reciprocal(s2, rsum)
            nc.vector.tensor_scalar_mul(s2, s2, SCALE)

            # C1t[d, l] = sum_s v^T exp1t  (unnormalized)
            c1t = []
            for j in range(DC):
                ps = psum.tile([128, 512], F32, name=f"c1_{g}_{j}", tag="mm512")
                for i in range(TS):
                    mm(
                        ps,
                        lhsT=v[i][:, 128 * j : 128 * (j + 1)],
                        rhs=e1t[i],
                        start=(i == 0),
                        stop=(i == TS - 1),
                    )
                c1t_j = work.tile(
                    [128, 512], BF16, name=f"c1t_{g}_{j}", tag=f"c1t{j}", bufs=2
                )
                nc.vector.tensor_copy(c1t_j, ps)
                c1t.append(c1t_j)

            # pass 2 per l-tile of 128
            for t in range(NT):
                row0 = 512 * g + 128 * t
                s2_t = s2[:, t : t + 1]
                # E2[l, s] in four 1-bank PSUM quarters (finer rotation)
                e2q = []
                for n in range(NS):
                    ps = psum.tile(
                        [128, 512], F32, name=f"e2_{g}_{t}_{n}", tag="e2q", bufs=4
                    )
                    for j in range(DC):
                        mm(
                            ps,
                            lhsT=c1t[j][:, 128 * t : 128 * (t + 1)],
                            rhs=kt[j][:, 512 * n : 512 * (n + 1)],
                            start=(j == 0),
                            stop=(j == DC - 1),
                        )
                    e2q.append(ps)

                nmax = work.tile([128, NS], F32, name=f"nmax_{g}_{t}", tag="nmax")
                for n in range(NS):
                    nc.vector.reduce_max(
                        nmax[:, n : n + 1], e2q[n], axis=mybir.AxisListType.X,
                        negate=True,
                    )
                bias2 = work.tile([128, 1], F32, name=f"bias2_{g}_{t}", tag="bias2")
                nc.vector.tensor_reduce(
                    bias2, nmax, axis=mybir.AxisListType.X, op=ALU.min
                )
                nc.vector.tensor_mul(bias2, bias2, s2_t)

                exp2 = work.tile([128, S], BF16, name=f"exp2_{g}_{t}", tag="exp2")
                sum2h = work.tile([128, NS], F32, name=f"sum2h_{g}_{t}", tag="sum2h")
                for n in range(NS):
                    nc.scalar.activation(
                        exp2[:, 512 * n : 512 * (n + 1)],
                        e2q[n],
                        AF.Exp,
                        bias=bias2,
                        scale=s2_t,
                        accum_out=sum2h[:, n : n + 1],
                    )
                recip2 = work.tile([128, 1], F32, name=f"recip2_{g}_{t}", tag="recip2")
                nc.vector.tensor_reduce(
                    recip2, sum2h, axis=mybir.AxisListType.X, op=ALU.add
                )
                nc.vector.reciprocal(recip2, recip2)

                # normalized A2 out (fp32)
                a2 = work.tile([128, S], F32, name=f"a2_{g}_{t}", tag="a2")
                nc.vector.tensor_scalar_mul(a2, exp2, recip2)
                nc.sync.dma_start(out=A_out[row0 : row0 + 128, :], in_=a2)

                # C2: one XBAR DMA transpose of exp2 -> [s, l] blocks
                a2t = work.tile(
                    [128, TS, 128], BF16, name=f"a2t_{g}_{t}", tag="a2t", bufs=2
                )
                nc.sync.dma_start(out=a2t, in_=exp2, transpose=True)

                c2ps = psum.tile([128, 512], F32, name=f"c2_{g}_{t}", tag="c2", bufs=1)
                for i in range(TS):
                    mm(
                        c2ps,
                        lhsT=a2t[:, i, :],
                        rhs=v[i],
                        start=(i == 0),
                        stop=(i == TS - 1),
                    )
                c2sb = work.tile([128, D], F32, name=f"c2sb_{g}_{t}", tag="c2sb")
                nc.scalar.activation(c2sb, c2ps, AF.Copy, scale=recip2)
                nc.sync.dma_start(out=C_out[row0 : row0 + 128, :], in_=c2sb)


_CACHE = {}


def _build(split_waits=True):
    key = ("nc", split_waits)
    if key not in _CACHE:
        nc = bass.Bass(
            "TRN2", target_bir_lowering=False, debug=False, num_devices=N_CORES
        )
        _emit(nc)
        if split_waits:
            _split_multi_waits(nc)
        _CACHE[key] = nc
    return _CACHE[key]


def kernel(H, K_w, K_b, V_w, V_b, Q_w):
    H = np.asarray(H, dtype=np.float32)
    K_w = np.asarray(K_w, dtype=np.float32)
    K_b = np.asarray(K_b, dtype=np.float32)
    V_w = np.asarray(V_w, dtype=np.float32)
    V_b = np.asarray(V_b, dtype=np.float32)
    Q_w = np.asarray(Q_w, dtype=np.float32)

    bf = ml_dtypes.bfloat16
    kwt = np.ascontiguousarray(K_w.T).astype(bf)  # [512, 512]
    vw_aug = np.concatenate([V_w.T, V_b[None, :]], axis=0).astype(bf)  # [513, 512]
    ones_row = np.ones((1, S), dtype=np.float32)

    in_maps = []
    for c in range(N_CORES):
        b, h = divmod(c, 2)
        h_aug = np.concatenate([H[b], ones_row], axis=0).astype(bf)  # [513, 2048]
        q_t = np.ascontiguousarray(Q_w[h * LH : (h + 1) * LH, :].T).astype(bf)
        in_maps.append(
            {"Haug": h_aug, "KWT": kwt, "Kb": K_b, "VWaug": vw_aug, "QT": q_t}
        )

    nc = _build()
    res = run_bass_kernel_spmd(nc, in_maps, core_ids=list(range(N_CORES)))

    C_full = np.empty((B, L, D), dtype=np.float32)
    A_full = np.empty((B, L, S), dtype=np.float32)
    for c in range(N_CORES):
        b, h = divmod(c, 2)
        C_full[b, h * LH : (h + 1) * LH, :] = res.results[c]["C"]
        A_full[b, h * LH : (h + 1) * LH, :] = res.results[c]["A"]
    return C_full, A_full
